# Initial kernel scaffold
#
"""DiffJPEG TRN2 Bass kernel.

Strategy (data-parallel over batch, 4 images per core on 8 cores):
separable blockwise DCT on natural image layout via block-diagonal
(16x) 8-point DCT matrices, with the RGB<->YCbCr color transforms folded
into the first/last matmul stages as PSUM-accumulated matmul groups.

Numerics: forward path (everything feeding the quantization round) runs
as exact-fp16-split matmuls (x = x1+x2, W = Wa+Wb, 3 accumulated terms
-> fp32-grade precision at 1 cycle/row). Rounding = (q+1.5*2^23)-1.5*2^23
on the DVE (bit-exact round-half-to-even, matching jnp.round). Inverse
path runs in plain fp16 (dequantized coefficients are exact integers*qt
<= 2047, exactly representable in fp16).

Pixel-domain affine offsets (-128, +-0.5 and the /255 rescale) are folded
into per-partition biases on PSUM evictions and into the stationary
matrices, which makes them exact w.r.t. the reference up to fp32 noise.
"""
import math
import numpy as np

_N_CORES = 8
_B = 32
_BPC = _B // _N_CORES  # images per core
_H = _W = 512
_NBAND = _H // 128

_state = {}


def _dct8_f64():
    D = np.zeros((8, 8), dtype=np.float64)
    for u in range(8):
        au = 1.0 / math.sqrt(2.0) if u == 0 else 1.0
        for x in range(8):
            D[u, x] = au * 0.5 * math.cos((2 * x + 1) * u * math.pi / 16.0)
    return D


def _y_quant_table():
    t = np.array([[16, 11, 10, 16, 24, 40, 51, 61], [12, 12, 14, 19, 26, 58, 60, 55],
                  [14, 13, 16, 24, 40, 57, 69, 56], [14, 17, 22, 29, 51, 87, 80, 62],
                  [18, 22, 37, 56, 68, 109, 103, 77], [24, 35, 55, 64, 81, 104, 113, 92],
                  [49, 64, 78, 87, 103, 121, 120, 101], [72, 92, 95, 98, 112, 100, 103, 99]],
                 dtype=np.float64).T
    return t


def _c_quant_table():
    t = np.full((8, 8), 99, dtype=np.float64)
    t[:4, :4] = np.array([[17, 18, 24, 47], [18, 21, 26, 66], [24, 26, 56, 99],
                          [47, 66, 99, 99]], dtype=np.float64).T
    return t


def _host_constants():
    D = _dct8_f64()
    Lb = np.kron(np.eye(16), D)          # [128,128] block-diag
    LbT = Lb.T

    # forward color (x255) coefficients: rows = (Y, Cb, Cr), cols = (R, G, B)
    MIX = np.array([
        [0.299 * 255, 0.587 * 255, 0.114 * 255],
        [-0.564 * 0.299 * 255, -0.564 * 0.587 * 255, 0.564 * (1 - 0.114) * 255],
        [0.713 * (1 - 0.299) * 255, -0.713 * 0.587 * 255, -0.713 * 0.114 * 255],
    ], dtype=np.float64)
    OFF = np.array([-128.0, -0.5, -0.5])
    # inverse color: rows = (R, G, B), cols = (Y', Cb', Cr')
    MI = np.array([[1.0, 0.0, 1.403], [1.0, -0.344, -0.714], [1.0, 1.773, 0.0]],
                  dtype=np.float64)

    def f16(a):
        return np.asarray(a, dtype=np.float16)

    def split16(M):
        a = f16(M)
        b = f16(M - a.astype(np.float64))
        return a, b

    # stage1 stationaries: lhsT = MIX[co,ci] * Lb^T, fp16-split pairs,
    # packed as [128, 9*128] in (co, ci) order.
    s1a = np.zeros((128, 9 * 128), dtype=np.float16)
    s1b = np.zeros((128, 9 * 128), dtype=np.float16)
    for co in range(3):
        for ci in range(3):
            a, b = split16(MIX[co, ci] * LbT)
            s1a[:, (co * 3 + ci) * 128:(co * 3 + ci + 1) * 128] = a
            s1b[:, (co * 3 + ci) * 128:(co * 3 + ci + 1) * 128] = b

    # stage3 stationary: lhsT = Lb^T, split pair packed [128, 256]
    l3a, l3b = split16(LbT)
    lb3 = np.concatenate([l3a, l3b], axis=1)

    # stage5 stationary: lhsT = Lb (single fp16)
    lb5 = f16(Lb)

    # stage7 stationaries: lhsT = MI[co,ci]/255 * Lb for nonzero MI,
    # packed [128, 7*128] in order of _S7_TERMS below.
    s7_terms = [(co, ci) for co in range(3) for ci in range(3) if MI[co, ci] != 0.0]
    s7 = np.zeros((128, len(s7_terms) * 128), dtype=np.float16)
    for k, (co, ci) in enumerate(s7_terms):
        s7[:, k * 128:(k + 1) * 128] = f16(MI[co, ci] / 255.0 * Lb)

    # quant pattern tiles in the transposed-frequency layout:
    # partition p = w-freq (v = p%8), free f = r-freq (u = f%8); value QT[u, v]
    QT = np.stack([_y_quant_table(), _c_quant_table(), _c_quant_table()])
    u = (np.arange(_W) % 8)[None, :]
    v = (np.arange(128) % 8)[:, None]
    qtt = np.zeros((3, 128, _W), dtype=np.float32)
    qti = np.zeros((3, 128, _W), dtype=np.float32)
    for c in range(3):
        pat = QT[c][u, v]
        qtt[c] = pat.astype(np.float32)
        qti[c] = (1.0 / pat).astype(np.float32)

    s0 = D[0].sum()  # 2*sqrt(2)
    bias1 = np.zeros((128, 3), dtype=np.float32)
    bias2 = np.zeros((128, 3), dtype=np.float32)
    kconst = np.array([128.0, 0.5, 0.5])
    for c in range(3):
        bias1[0::8, c] = np.float32(OFF[c] * s0)
        bias2[0::8, c] = np.float32(kconst[c] * s0)

    ident = np.eye(128, dtype=np.float16)

    return dict(s1a=s1a, s1b=s1b, lb3=lb3, lb5=lb5, s7=s7, qtt=qtt, qti=qti,
                bias1=bias1, bias2=bias2, ident=ident), s7_terms, MI


def _build_program():
    import sys
    if "/opt/trn_rl_repo" not in sys.path:
        sys.path.insert(0, "/opt/trn_rl_repo")
    from contextlib import ExitStack
    import concourse.bacc as bacc
    import concourse.tile as tile
    from concourse import mybir
    from concourse.alu_op_type import AluOpType
    import bass_rust

    ACT_ID = bass_rust.ActivationFunctionType.Identity
    F32 = mybir.dt.float32
    F16 = mybir.dt.float16
    CMAGIC = float(np.float32(1.5 * 2 ** 23))

    consts, s7_terms, MI = _host_constants()

    nc = bacc.Bacc("TRN2", target_bir_lowering=False, debug=False,
                   num_devices=_N_CORES)

    x1 = nc.declare_dram_parameter("x1", [_BPC, 3, _H, _W], F16, isOutput=False)
    x2 = nc.declare_dram_parameter("x2", [_BPC, 3, _H, _W], F16, isOutput=False)
    cs = {}
    for name, arr in consts.items():
        dt = F16 if arr.dtype == np.float16 else F32
        cs[name] = nc.declare_dram_parameter(name, list(arr.shape), dt,
                                             isOutput=False)
    out = nc.declare_dram_parameter("out", [_BPC, 3, _H, _W], F32, isOutput=True)

    with tile.TileContext(nc) as tc, ExitStack() as ctx:
        cpool = ctx.enter_context(tc.tile_pool(name="consts", bufs=1))
        xin = ctx.enter_context(tc.tile_pool(name="xin", bufs=30))
        apool = ctx.enter_context(tc.tile_pool(name="apool", bufs=26))
        atp = ctx.enter_context(tc.tile_pool(name="atp", bufs=6))
        qpool = ctx.enter_context(tc.tile_pool(name="qpool", bufs=3))
        dqpool = ctx.enter_context(tc.tile_pool(name="dqpool", bufs=3))
        fpool = ctx.enter_context(tc.tile_pool(name="fpool", bufs=26))
        gpool = ctx.enter_context(tc.tile_pool(name="gpool", bufs=7))
        opool = ctx.enter_context(tc.tile_pool(name="opool", bufs=4))
        ps1 = ctx.enter_context(tc.tile_pool(name="ps1", bufs=2, space="PSUM"))
        pst1 = ctx.enter_context(tc.tile_pool(name="pst1", bufs=2, space="PSUM"))
        ps3 = ctx.enter_context(tc.tile_pool(name="ps3", bufs=1, space="PSUM"))
        ps5 = ctx.enter_context(tc.tile_pool(name="ps5", bufs=1, space="PSUM"))
        pst2 = ctx.enter_context(tc.tile_pool(name="pst2", bufs=1, space="PSUM"))
        ps7 = ctx.enter_context(tc.tile_pool(name="ps7", bufs=1, space="PSUM"))

        # --- load constants ---
        ct = {}
        for name, arr in consts.items():
            dt = F16 if arr.dtype == np.float16 else F32
            t = cpool.tile(list(arr.shape), dt, tag=f"c_{name}")
            nc.sync.dma_start(t[:], cs[name][:])
            ct[name] = t

        def s1w(half, co, ci):
            t = ct["s1a"] if half == 0 else ct["s1b"]
            k = co * 3 + ci
            return t[:, k * 128:(k + 1) * 128]

        def lb3w(half):
            return ct["lb3"][:, half * 128:(half + 1) * 128]

        def s7w(k):
            return ct["s7"][:, k * 128:(k + 1) * 128]

        for img in range(_BPC):
            # ---- load input tiles (fp16 split pair) ----
            xt = {}
            for ci in range(3):
                for b in range(_NBAND):
                    t1 = xin.tile([128, _W], F16, tag="x")
                    nc.sync.dma_start(t1[:], x1[img, ci, b * 128:(b + 1) * 128, :])
                    t2 = xin.tile([128, _W], F16, tag="x")
                    nc.sync.dma_start(t2[:], x2[img, ci, b * 128:(b + 1) * 128, :])
                    xt[ci, b] = (t1, t2)

            # ---- stage 1: color + vertical DCT ----
            A = {}
            for b in range(_NBAND):
                for co in range(3):
                    ps = ps1.tile([128, _W], F32, tag="s1")
                    for k, ci in enumerate(range(3)):
                        nc.tensor.matmul(ps[:], s1w(0, co, ci), xt[ci, b][0][:],
                                         start=(k == 0), stop=False)
                        nc.tensor.matmul(ps[:], s1w(0, co, ci), xt[ci, b][1][:],
                                         start=False, stop=False)
                        nc.tensor.matmul(ps[:], s1w(1, co, ci), xt[ci, b][0][:],
                                         start=False, stop=(k == 2))
                    a1 = apool.tile([128, _W], F16, tag="a")
                    nc.scalar.activation(a1[:], ps[:], ACT_ID,
                                         bias=ct["bias1"][:, co:co + 1], scale=1.0)
                    a2 = apool.tile([128, _W], F16, tag="a")
                    nc.vector.scalar_tensor_tensor(
                        a2[:], ps[:], ct["bias1"][:, co:co + 1], a1[:],
                        op0=AluOpType.add, op1=AluOpType.subtract)
                    A[co, b] = (a1, a2)

            # ---- T1 + stage3 + quant + stage5, per (channel, w-band) ----
            Fv = {}
            for co in range(3):
                for w in range(_NBAND):
                    pa = pst1.tile([128, _W], F16, tag="t1")
                    pb = pst1.tile([128, _W], F16, tag="t1")
                    for b in range(_NBAND):
                        nc.tensor.transpose(pa[:, b * 128:(b + 1) * 128],
                                            A[co, b][0][:, w * 128:(w + 1) * 128],
                                            ct["ident"][:])
                        nc.tensor.transpose(pb[:, b * 128:(b + 1) * 128],
                                            A[co, b][1][:, w * 128:(w + 1) * 128],
                                            ct["ident"][:])
                    at1 = atp.tile([128, _W], F16, tag="at")
                    nc.scalar.copy(at1[:], pa[:])
                    at2 = atp.tile([128, _W], F16, tag="at")
                    nc.scalar.copy(at2[:], pb[:])

                    ps = ps3.tile([128, _W], F32, tag="s3")
                    nc.tensor.matmul(ps[:], lb3w(0), at1[:], start=True, stop=False)
                    nc.tensor.matmul(ps[:], lb3w(0), at2[:], start=False, stop=False)
                    nc.tensor.matmul(ps[:], lb3w(1), at1[:], start=False, stop=True)

                    q = qpool.tile([128, _W], F32, tag="q")
                    nc.vector.tensor_tensor(q[:], ps[:], ct["qti"][co],
                                            op=AluOpType.mult)
                    rq = qpool.tile([128, _W], F32, tag="rq")
                    nc.vector.tensor_scalar(rq[:], q[:], CMAGIC, -CMAGIC,
                                            op0=AluOpType.add, op1=AluOpType.add)
                    dq = dqpool.tile([128, _W], F16, tag="dq")
                    nc.vector.tensor_tensor(dq[:], rq[:], ct["qtt"][co],
                                            op=AluOpType.mult)

                    pf = ps5.tile([128, _W], F32, tag="s5")
                    nc.tensor.matmul(pf[:], ct["lb5"][:], dq[:], start=True,
                                     stop=True)
                    f = fpool.tile([128, _W], F16, tag="f")
                    nc.scalar.copy(f[:], pf[:])
                    Fv[co, w] = f

            # ---- T2 + stage7 + clip + store, per row-band ----
            for b in range(_NBAND):
                G = {}
                for ci in range(3):
                    pg = pst2.tile([128, _W], F16, tag="t2")
                    for w in range(_NBAND):
                        nc.tensor.transpose(pg[:, w * 128:(w + 1) * 128],
                                            Fv[ci, w][:, b * 128:(b + 1) * 128],
                                            ct["ident"][:])
                    g = gpool.tile([128, _W], F16, tag="g")
                    nc.scalar.activation(g[:], pg[:], ACT_ID,
                                         bias=ct["bias2"][:, ci:ci + 1], scale=1.0)
                    G[ci] = g
                for co in range(3):
                    terms = [k for k, (tco, _) in enumerate(s7_terms) if tco == co]
                    po = ps7.tile([128, _W], F32, tag="s7")
                    for j, k in enumerate(terms):
                        ci = s7_terms[k][1]
                        nc.tensor.matmul(po[:], s7w(k), G[ci][:],
                                         start=(j == 0), stop=(j == len(terms) - 1))
                    ot = opool.tile([128, _W], F32, tag="o")
                    nc.vector.tensor_scalar(ot[:], po[:], 0.0, 1.0,
                                            op0=AluOpType.max, op1=AluOpType.min)
                    nc.sync.dma_start(out[img, co, b * 128:(b + 1) * 128, :], ot[:])

    nc.compile()
    return nc, consts


def _get_program():
    if "nc" not in _state:
        nc, consts = _build_program()
        _state["nc"] = nc
        _state["consts"] = consts
    return _state["nc"], _state["consts"]


def kernel(image: np.ndarray) -> np.ndarray:
    import sys
    if "/opt/trn_rl_repo" not in sys.path:
        sys.path.insert(0, "/opt/trn_rl_repo")
    from concourse.bass_utils import run_bass_kernel_spmd

    image = np.asarray(image)
    assert image.shape == (_B, 3, _H, _W), image.shape
    nc, consts = _get_program()

    img32 = image.astype(np.float32, copy=False)
    x1 = img32.astype(np.float16)
    x2 = (img32 - x1.astype(np.float32)).astype(np.float16)

    in_maps = []
    for c in range(_N_CORES):
        sl = slice(c * _BPC, (c + 1) * _BPC)
        m = dict(x1=x1[sl], x2=x2[sl])
        m.update(consts)
        in_maps.append(m)

    res = run_bass_kernel_spmd(nc, in_maps, core_ids=list(range(_N_CORES)))
    outs = [res.results[c]["out"] for c in range(_N_CORES)]
    return np.concatenate(outs, axis=0).astype(np.float32)


if __name__ == "__main__":
    rng = np.random.default_rng(0)
    img = rng.uniform(size=(_B, 3, _H, _W)).astype(np.float32)
    out = kernel(img)
    print(out.shape, out.dtype, float(out.min()), float(out.max()))


# revision 13
# speedup vs baseline: 2.0518x; 2.0518x over previous
"""DiffJPEG TRN2 Bass kernel.

Strategy (data-parallel over batch, 4 images per core on 8 cores):
separable blockwise DCT on natural image layout via block-diagonal
(16x) 8-point DCT matrices, with the RGB<->YCbCr color transforms folded
into the first/last matmul stages as PSUM-accumulated matmul groups.

Numerics: forward path (everything feeding the quantization round) runs
as exact-fp16-split matmuls (x = x1+x2, W = Wa+Wb, 3 accumulated terms
-> fp32-grade precision at 1 cycle/row). Rounding = (q+1.5*2^23)-1.5*2^23
on the DVE (bit-exact round-half-to-even, matching jnp.round). Inverse
path runs in plain fp16 (dequantized coefficients are exact integers*qt
<= 2047, exactly representable in fp16).

Pixel-domain affine offsets (-128, +-0.5 and the /255 rescale) are folded
into per-partition biases on PSUM evictions and into the stationary
matrices, which makes them exact w.r.t. the reference up to fp32 noise.
"""
import math
import numpy as np

_N_CORES = 8
_B = 32
_BPC = _B // _N_CORES  # images per core
_H = _W = 512
_NBAND = _H // 128

_state = {}


def _dct8_f64():
    D = np.zeros((8, 8), dtype=np.float64)
    for u in range(8):
        au = 1.0 / math.sqrt(2.0) if u == 0 else 1.0
        for x in range(8):
            D[u, x] = au * 0.5 * math.cos((2 * x + 1) * u * math.pi / 16.0)
    return D


def _y_quant_table():
    t = np.array([[16, 11, 10, 16, 24, 40, 51, 61], [12, 12, 14, 19, 26, 58, 60, 55],
                  [14, 13, 16, 24, 40, 57, 69, 56], [14, 17, 22, 29, 51, 87, 80, 62],
                  [18, 22, 37, 56, 68, 109, 103, 77], [24, 35, 55, 64, 81, 104, 113, 92],
                  [49, 64, 78, 87, 103, 121, 120, 101], [72, 92, 95, 98, 112, 100, 103, 99]],
                 dtype=np.float64).T
    return t


def _c_quant_table():
    t = np.full((8, 8), 99, dtype=np.float64)
    t[:4, :4] = np.array([[17, 18, 24, 47], [18, 21, 26, 66], [24, 26, 56, 99],
                          [47, 66, 99, 99]], dtype=np.float64).T
    return t


def _host_constants():
    D = _dct8_f64()
    Lb = np.kron(np.eye(16), D)          # [128,128] block-diag
    LbT = Lb.T

    # forward color (x255) coefficients: rows = (Y, Cb, Cr), cols = (R, G, B)
    MIX = np.array([
        [0.299 * 255, 0.587 * 255, 0.114 * 255],
        [-0.564 * 0.299 * 255, -0.564 * 0.587 * 255, 0.564 * (1 - 0.114) * 255],
        [0.713 * (1 - 0.299) * 255, -0.713 * 0.587 * 255, -0.713 * 0.114 * 255],
    ], dtype=np.float64)
    OFF = np.array([-128.0, -0.5, -0.5])
    # inverse color: rows = (R, G, B), cols = (Y', Cb', Cr')
    MI = np.array([[1.0, 0.0, 1.403], [1.0, -0.344, -0.714], [1.0, 1.773, 0.0]],
                  dtype=np.float64)

    def f16(a):
        return np.asarray(a, dtype=np.float16)

    def split16(M):
        a = f16(M)
        b = f16(M - a.astype(np.float64))
        return a, b

    # stage1 stationaries: lhsT = MIX[co,ci] * Lb^T, fp16-split pairs,
    # packed as [128, 9*128] in (co, ci) order.
    s1a = np.zeros((128, 9 * 128), dtype=np.float16)
    s1b = np.zeros((128, 9 * 128), dtype=np.float16)
    for co in range(3):
        for ci in range(3):
            a, b = split16(MIX[co, ci] * LbT)
            s1a[:, (co * 3 + ci) * 128:(co * 3 + ci + 1) * 128] = a
            s1b[:, (co * 3 + ci) * 128:(co * 3 + ci + 1) * 128] = b

    # stage3 stationary: lhsT = Lb^T, split pair packed [128, 256]
    l3a, l3b = split16(LbT)
    lb3 = np.concatenate([l3a, l3b], axis=1)

    # stage5 stationary: lhsT = Lb (single fp16)
    lb5 = f16(Lb)

    # stage7 stationaries: lhsT = MI[co,ci]/255 * Lb for nonzero MI,
    # packed [128, 7*128] in order of _S7_TERMS below.
    s7_terms = [(co, ci) for co in range(3) for ci in range(3) if MI[co, ci] != 0.0]
    s7 = np.zeros((128, len(s7_terms) * 128), dtype=np.float16)
    for k, (co, ci) in enumerate(s7_terms):
        s7[:, k * 128:(k + 1) * 128] = f16(MI[co, ci] / 255.0 * Lb)

    # quant pattern tiles in the transposed-frequency layout:
    # partition p = w-freq (v = p%8), free f = r-freq (u = f%8); value QT[u, v]
    QT = np.stack([_y_quant_table(), _c_quant_table(), _c_quant_table()])
    u = (np.arange(_W) % 8)[None, :]
    v = (np.arange(128) % 8)[:, None]
    qtt = np.zeros((3, 128, _W), dtype=np.float32)
    qti = np.zeros((3, 128, _W), dtype=np.float32)
    for c in range(3):
        pat = QT[c][u, v]
        qtt[c] = pat.astype(np.float32)
        qti[c] = (1.0 / pat).astype(np.float32)

    s0 = D[0].sum()  # 2*sqrt(2)
    bias1 = np.zeros((128, 3), dtype=np.float32)
    bias2 = np.zeros((128, 3), dtype=np.float32)
    kconst = np.array([128.0, 0.5, 0.5])
    for c in range(3):
        bias1[0::8, c] = np.float32(OFF[c] * s0)
        bias2[0::8, c] = np.float32(kconst[c] * s0)

    ident = np.eye(128, dtype=np.float16)

    return dict(s1a=s1a, s1b=s1b, lb3=lb3, lb5=lb5, s7=s7, qtt=qtt, qti=qti,
                bias1=bias1, bias2=bias2, ident=ident), s7_terms, MI


def _build_program(repeat: int = 1):
    import sys
    if "/opt/trn_rl_repo" not in sys.path:
        sys.path.insert(0, "/opt/trn_rl_repo")
    from contextlib import ExitStack
    import concourse.bacc as bacc
    import concourse.tile as tile
    from concourse import mybir
    from concourse.alu_op_type import AluOpType
    import bass_rust

    ACT_ID = bass_rust.ActivationFunctionType.Identity
    F32 = mybir.dt.float32
    F16 = mybir.dt.float16
    CMAGIC = float(np.float32(1.5 * 2 ** 23))

    consts, s7_terms, MI = _host_constants()

    nc = bacc.Bacc("TRN2", target_bir_lowering=False, debug=False,
                   num_devices=_N_CORES)

    x1 = nc.declare_dram_parameter("x1", [_BPC, 3, _H, _W], F16, isOutput=False)
    x2 = nc.declare_dram_parameter("x2", [_BPC, 3, _H, _W], F16, isOutput=False)
    cs = {}
    for name, arr in consts.items():
        dt = F16 if arr.dtype == np.float16 else F32
        cs[name] = nc.declare_dram_parameter(name, list(arr.shape), dt,
                                             isOutput=False)
    out = nc.declare_dram_parameter("out", [_BPC, 3, _H, _W], F32, isOutput=True)

    with tile.TileContext(nc) as tc, ExitStack() as ctx:
        cpool = ctx.enter_context(tc.tile_pool(name="consts", bufs=1))
        xin = ctx.enter_context(tc.tile_pool(name="xin", bufs=48))
        apool = ctx.enter_context(tc.tile_pool(name="apool", bufs=48))
        atp = ctx.enter_context(tc.tile_pool(name="atp", bufs=6))
        qpool = ctx.enter_context(tc.tile_pool(name="qpool", bufs=3))
        dqpool = ctx.enter_context(tc.tile_pool(name="dqpool", bufs=14))
        fpool = ctx.enter_context(tc.tile_pool(name="fpool", bufs=24))
        gpool = ctx.enter_context(tc.tile_pool(name="gpool", bufs=14))
        opool = ctx.enter_context(tc.tile_pool(name="opool", bufs=4))
        ps1 = ctx.enter_context(tc.tile_pool(name="ps1", bufs=2, space="PSUM"))
        ps3 = ctx.enter_context(tc.tile_pool(name="ps3", bufs=2, space="PSUM"))
        ps5 = ctx.enter_context(tc.tile_pool(name="ps5", bufs=1, space="PSUM"))
        ps7 = ctx.enter_context(tc.tile_pool(name="ps7", bufs=1, space="PSUM"))
        psT = ctx.enter_context(tc.tile_pool(name="psT", bufs=2, space="PSUM"))

        # --- load constants (stage-1-critical ones first) ---
        ct = {}
        _order = ["s1a", "s1b", "bias1", "ident", "lb3", "qti", "qtt",
                  "lb5", "s7", "bias2"]
        consts_ordered = {k: consts[k] for k in _order}
        for name, arr in consts_ordered.items():
            dt = F16 if arr.dtype == np.float16 else F32
            if name in ("qtt", "qti"):
                t = cpool.tile([128, 3, _W], dt, tag=f"c_{name}")
                for c in range(3):
                    nc.sync.dma_start(t[:, c, :], cs[name][c])
            else:
                t = cpool.tile(list(arr.shape), dt, tag=f"c_{name}")
                nc.sync.dma_start(t[:], cs[name][:])
            ct[name] = t

        def s1w(half, co, ci):
            t = ct["s1a"] if half == 0 else ct["s1b"]
            k = co * 3 + ci
            return t[:, k * 128:(k + 1) * 128]

        def lb3w(half):
            return ct["lb3"][:, half * 128:(half + 1) * 128]

        def s7w(k):
            return ct["s7"][:, k * 128:(k + 1) * 128]

        def _load_img(img):
            xt = {}
            for ci in range(3):
                for b in range(_NBAND):
                    t1 = xin.tile([128, _W], F16, tag="x")
                    nc.sync.dma_start(t1[:], x1[img, ci, b * 128:(b + 1) * 128, :])
                    t2 = xin.tile([128, _W], F16, tag="x")
                    nc.sync.dma_start(t2[:], x2[img, ci, b * 128:(b + 1) * 128, :])
                    xt[ci, b] = (t1, t2)
            return xt

        xt_next = _load_img(0)
        for rep in range(repeat):
          for img in range(_BPC):
            xt = xt_next

            # ---- stage 1: color + vertical DCT ----
            A = {}
            for b in range(_NBAND):
                for co in range(3):
                    ps = ps1.tile([128, _W], F32, tag="s1")
                    for k, ci in enumerate(range(3)):
                        nc.tensor.matmul(ps[:], s1w(0, co, ci), xt[ci, b][0][:],
                                         start=(k == 0), stop=False)
                        nc.tensor.matmul(ps[:], s1w(0, co, ci), xt[ci, b][1][:],
                                         start=False, stop=False)
                        nc.tensor.matmul(ps[:], s1w(1, co, ci), xt[ci, b][0][:],
                                         start=False, stop=(k == 2))
                    a1 = apool.tile([128, _W], F16, tag="a")
                    nc.scalar.activation(a1[:], ps[:], ACT_ID,
                                         bias=ct["bias1"][:, co:co + 1], scale=1.0)
                    a2 = apool.tile([128, _W], F16, tag="a")
                    nc.vector.scalar_tensor_tensor(
                        a2[:], ps[:], ct["bias1"][:, co:co + 1], a1[:],
                        op0=AluOpType.add, op1=AluOpType.subtract)
                    A[co, b] = (a1, a2)

            if img + 1 < _BPC:
                xt_next = _load_img(img + 1)
            elif rep + 1 < repeat:
                xt_next = _load_img(0)

            # ---- T1 + stage3 + quant, per (channel, w-band), skewed ----
            tiles = [(co, w) for co in range(3) for w in range(_NBAND)]
            DQ = {}
            pend = None

            def _t1(co, w):
                pt = psT.tile([128, 2 * _W], F16, tag="tps")
                pa = pt[:, 0:_W]
                pb = pt[:, _W:2 * _W]
                for b in range(_NBAND):
                    nc.tensor.transpose(pa[:, b * 128:(b + 1) * 128],
                                        A[co, b][0][:, w * 128:(w + 1) * 128],
                                        ct["ident"][:])
                    nc.tensor.transpose(pb[:, b * 128:(b + 1) * 128],
                                        A[co, b][1][:, w * 128:(w + 1) * 128],
                                        ct["ident"][:])
                at1 = atp.tile([128, _W], F16, tag="at")
                nc.scalar.copy(at1[:], pa[:])
                at2 = atp.tile([128, _W], F16, tag="at")
                nc.vector.tensor_copy(at2[:], pb[:])
                return at1, at2

            def _s3(co, w, at1, at2):
                ps = ps3.tile([128, _W], F32, tag="s3")
                nc.tensor.matmul(ps[:], lb3w(0), at1[:], start=True, stop=False)
                nc.tensor.matmul(ps[:], lb3w(0), at2[:], start=False, stop=False)
                nc.tensor.matmul(ps[:], lb3w(1), at1[:], start=False, stop=True)
                q = qpool.tile([128, _W], F32, tag="q")
                nc.vector.tensor_tensor(q[:], ps[:], ct["qti"][:, co, :],
                                        op=AluOpType.mult)
                rq = qpool.tile([128, _W], F32, tag="rq")
                nc.vector.tensor_scalar(rq[:], q[:], CMAGIC, -CMAGIC,
                                        op0=AluOpType.add, op1=AluOpType.add)
                dq = dqpool.tile([128, _W], F16, tag="dq")
                nc.gpsimd.tensor_tensor(dq[:], rq[:], ct["qtt"][:, co, :],
                                        op=AluOpType.mult)
                DQ[co, w] = dq

            for co, w in tiles:
                ats = _t1(co, w)
                if pend is not None:
                    _s3(*pend)
                pend = (co, w, *ats)
            _s3(*pend)

            # ---- stage5 (vertical IDCT on transposed layout) ----
            Fv = {}
            for co, w in tiles:
                pf = ps5.tile([128, _W], F32, tag="s5")
                nc.tensor.matmul(pf[:], ct["lb5"][:], DQ[co, w][:], start=True,
                                 stop=True)
                f = fpool.tile([128, _W], F16, tag="f")
                nc.scalar.copy(f[:], pf[:])
                Fv[co, w] = f

            # ---- T2 + stage7, skewed per row-band ----
            G = {}

            def _t2(b):
                for ci in range(3):
                    ptg = psT.tile([128, 2 * _W], F16, tag="tps")
                    pg = ptg[:, 0:_W]
                    for w in range(_NBAND):
                        nc.tensor.transpose(pg[:, w * 128:(w + 1) * 128],
                                            Fv[ci, w][:, b * 128:(b + 1) * 128],
                                            ct["ident"][:])
                    g = gpool.tile([128, _W], F16, tag="g")
                    nc.scalar.activation(g[:], pg[:], ACT_ID,
                                         bias=ct["bias2"][:, ci:ci + 1], scale=1.0)
                    G[ci, b] = g

            def _s7(b):
                for co in range(3):
                    terms = [k for k, (tco, _) in enumerate(s7_terms) if tco == co]
                    po = ps7.tile([128, _W], F32, tag="s7")
                    for j, k in enumerate(terms):
                        ci = s7_terms[k][1]
                        nc.tensor.matmul(po[:], s7w(k), G[ci, b][:],
                                         start=(j == 0), stop=(j == len(terms) - 1))
                    ot = opool.tile([128, _W], F32, tag="o")
                    nc.vector.tensor_scalar(ot[:], po[:], 0.0, 1.0,
                                            op0=AluOpType.max, op1=AluOpType.min)
                    nc.sync.dma_start(out[img, co, b * 128:(b + 1) * 128, :], ot[:])

            _t2(0)
            for b in range(1, _NBAND):
                _t2(b)
                _s7(b - 1)
            _s7(_NBAND - 1)

    nc.compile()
    return nc, consts


def _get_program(repeat: int = 1):
    key = ("nc", repeat)
    if key not in _state:
        nc, consts = _build_program(repeat)
        _state[key] = (nc, consts)
    return _state[key]


def kernel(image: np.ndarray) -> np.ndarray:
    import sys
    if "/opt/trn_rl_repo" not in sys.path:
        sys.path.insert(0, "/opt/trn_rl_repo")
    from concourse.bass_utils import run_bass_kernel_spmd

    image = np.asarray(image)
    assert image.shape == (_B, 3, _H, _W), image.shape
    nc, consts = _get_program()

    img32 = image.astype(np.float32, copy=False)
    x1 = img32.astype(np.float16)
    x2 = (img32 - x1.astype(np.float32)).astype(np.float16)

    in_maps = []
    for c in range(_N_CORES):
        sl = slice(c * _BPC, (c + 1) * _BPC)
        m = dict(x1=x1[sl], x2=x2[sl])
        m.update(consts)
        in_maps.append(m)

    res = run_bass_kernel_spmd(nc, in_maps, core_ids=list(range(_N_CORES)))
    _state["exec_time_ns"] = getattr(res, "exec_time_ns", None)
    _state["profile_json"] = getattr(res, "profile_json", None)
    outs = [res.results[c]["out"] for c in range(_N_CORES)]
    return np.concatenate(outs, axis=0).astype(np.float32)


if __name__ == "__main__":
    rng = np.random.default_rng(0)
    img = rng.uniform(size=(_B, 3, _H, _W)).astype(np.float32)
    out = kernel(img)
    print(out.shape, out.dtype, float(out.min()), float(out.max()))
